# revision 1
# baseline (speedup 1.0000x reference)
"""Trainium2 Bass kernel for batch-4096 LSTM (H=32, I=1, T=512) + linear head.

Strategy: data-parallel over 8 NeuronCores (512 batch rows each). Each core
runs the full T=512 sequential scan with the batch on the matmul free dim,
split into G=3 independent batch chains (176/168/168 columns) that
interleave to hide per-step cross-engine semaphore latency.

  state tile  [64, BCq] f16 : rows 0-31 = h_t, rows 32-63 = x window (32 steps)
  lhsT        [64, 128] f16 : rows 0-31 = W_hh.T, row 32+j = W_ih row (step j
                              of the window), zeros elsewhere. 32 variants per
                              band pattern.
  psum gates  [128, BCq] f32: per-pattern gate order (A: [f,i,o,g], B: [i,f,g,o])

Per-step math (all four gates through ONE sigmoid ACT op):
  g-gate rows of W and bias are pre-scaled by 2, so s_g = sigmoid(2 a_g) and
  g'' = s_g - 1/2 = tanh(a_g)/2.  Tracking c' = c/2:
     c' = f * c' + i * g''         (exactly c/2)
     h  = o * tanh(2 c')           (ACT scale=2 immediate)
Bias (b_ih+b_hh) is applied via the ACT per-partition bias operand.

DVE tensor op INPUTS must share a start partition (outputs may shift); the
band patterns keep every product band-consistent:
  pattern A: f@0, i@32, o@64, g@96 ; c'@0,  g''@32, t1@0,  th@64
  pattern B: i@0, f@32, g@64, o@96 ; c'@32, g''@0,  t1@32, th@96
Elementwise tensors (s, g'', t1, c', th) are fp16 for DVE 2x mode; PSUM, the
bias and the output head are fp32 (the head has ~10x cancellation
amplification, so the last step produces a fp32 h for a fp32 head matmul).
"""

import os
import numpy as np

H = 32
T = 512
BTOT = 4096
NCORES = 8
B = BTOT // NCORES  # 512 per core
G = int(os.environ.get("G", "3"))   # independent batch chains per core
if G == 2:
    CSZ = [256, 256]
elif G == 3:
    CSZ = [176, 168, 168]
elif G == 4:
    CSZ = [128, 128, 128, 128]
else:
    raise ValueError(G)
COFF = [sum(CSZ[:q]) for q in range(G)]
PAT = [q % 2 for q in range(G)]     # band pattern per chain
NW = 32             # x-window steps held in state rows 32..63
K = 32 + NW         # matmul contraction dim

_cache: dict = {}

# engine assignment per chain: ops on gpsimd ("g") or vector ("v")
ENG = [
    {
        "t1": os.environ.get(f"ENG{q}_T1", "v"),
        "c": os.environ.get(f"ENG{q}_C", "v"),
        "gc": os.environ.get(f"ENG{q}_GC", "v"),
    }
    for q in range(G)
]
WORK_BUFS = int(os.environ.get("WORK_BUFS", "4"))
PSUM_BUFS = int(os.environ.get("PSUM_BUFS", "2"))
EMIT = os.environ.get("EMIT", "chain")
PINGPONG = os.environ.get("PINGPONG", "1") == "1"
MMSPLIT = os.environ.get("MMSPLIT", "0") == "1"
XCOPY_ENG = os.environ.get("XCOPY_ENG", "g")
DUMMY_MM = int(os.environ.get("DUMMY_MM", "0"))

_I, _F, _G_, _O = (
    np.arange(0, 32),
    np.arange(32, 64),
    np.arange(64, 96),
    np.arange(96, 128),
)  # PyTorch row blocks [i, f, g, o]
PERMS = [
    np.concatenate([_F, _I, _O, _G_]),  # pattern A
    np.concatenate([_I, _F, _G_, _O]),  # pattern B
]
BAND = [
    dict(f=0, i=32, o=64, g=96),  # pattern A
    dict(i=0, f=32, g=64, o=96),  # pattern B
]


def _build(t_steps=T):
    """Build + compile the per-core Bass program (same NEFF for all cores)."""
    from contextlib import ExitStack
    import concourse.tile as tile
    from concourse import bacc, mybir
    from concourse.bass import ts

    f32 = mybir.dt.float32
    f16 = mybir.dt.float16
    AF = mybir.ActivationFunctionType
    OP = mybir.AluOpType

    nc = bacc.Bacc(
        "TRN2", target_bir_lowering=False, debug=False, num_devices=NCORES
    )
    xT_d = nc.dram_tensor("xT", [T, B], f16, kind="ExternalInput").ap()
    wvar_d = nc.dram_tensor("wvar", [K, 2 * NW * 128], f16, kind="ExternalInput").ap()
    bias_d = nc.dram_tensor("bias", [128, 2], f32, kind="ExternalInput").ap()
    wout_d = nc.dram_tensor("wout", [H, 1], f32, kind="ExternalInput").ap()
    bout_d = nc.dram_tensor("bout", [1, 1], f32, kind="ExternalInput").ap()
    out_d = nc.dram_tensor("out", [1, B], f32, kind="ExternalOutput").ap()
    scr_d = None
    if DUMMY_MM:
        scr_d = nc.dram_tensor("scr", [1, 128], f32, kind="ExternalOutput").ap()

    with tile.TileContext(nc) as tc:
        with ExitStack() as ctx:
            const = ctx.enter_context(tc.tile_pool(name="const", bufs=1))
            spool = ctx.enter_context(tc.tile_pool(name="state", bufs=1))
            work = ctx.enter_context(tc.tile_pool(name="work", bufs=WORK_BUFS))
            psum = ctx.enter_context(
                tc.tile_pool(name="psum", bufs=PSUM_BUFS, space="PSUM")
            )

            xT_sb = [
                const.tile([128, B], f16, tag=f"xT{i}", name=f"xT{i}")
                for i in range(4)
            ]
            nc.sync.dma_start(xT_sb[0][0:32, :], xT_d[0:32, :])
            nc.sync.dma_start(xT_sb[0][32:128, :], xT_d[32:128, :])
            for i in range(1, 4):
                nc.sync.dma_start(xT_sb[i][:], xT_d[ts(i, 128), :])
            wvar_sb = const.tile([K, 2 * NW * 128], f16, tag="wvar")
            wchunk = 2 * NW * 128 // 8
            for i in range(8):
                nc.sync.dma_start(wvar_sb[:, ts(i, wchunk)], wvar_d[:, ts(i, wchunk)])
            bias_sb = const.tile([128, 2], f32, tag="bias")
            nc.sync.dma_start(bias_sb[:], bias_d[:])
            wout_sb = const.tile([H, 1], f32, tag="wout")
            nc.sync.dma_start(wout_sb[:], wout_d[:])
            bout_sb = const.tile([1, 1], f32, tag="bout")
            nc.sync.dma_start(bout_sb[:], bout_d[:])

            st, ctsl = [], []
            for q in range(G):
                stq = spool.tile([K, CSZ[q]], f16, tag=f"st{q}", name=f"st{q}")
                ct0 = spool.tile([64, CSZ[q]], f16, tag=f"ci{q}", name=f"ci{q}")
                cb = 32 * PAT[q]
                st.append(stq)
                ctsl.append(ct0[cb : cb + 32, :])
                nc.vector.memset(stq[0:32, :], 0.0)
                nc.vector.memset(ctsl[q][:], 0.0)

            hfs = {}
            for t in range(t_steps):
                j = t % NW
                if j == 0:
                    ti, p0 = t // 128, t % 128
                    e_x = nc.gpsimd if XCOPY_ENG == "g" else nc.vector
                    for q in range(G):
                        e_x.tensor_copy(
                            st[q][32:64, :],
                            xT_sb[ti][p0 : p0 + 32, COFF[q] : COFF[q] + CSZ[q]],
                        )
                sv, t1v, thv = {}, {}, {}
                if DUMMY_MM:
                    dps = psum.tile([128, 128], f32, tag="dps", name="dps", bufs=1)
                    for _d in range(DUMMY_MM):
                        nc.tensor.matmul(
                            dps[:],
                            wvar_sb[:, 0:128],
                            wvar_sb[:, 128:256],
                            start=(_d == 0 and t == 0),
                            stop=(_d == DUMMY_MM - 1 and t == t_steps - 1),
                            skip_group_check=True,
                        )

                def ph_mm(q):
                    p = PAT[q]
                    bc = CSZ[q]
                    ps = psum.tile([128, bc], f32, tag=f"ps{q}", name=f"ps{q}")
                    if MMSPLIT:
                        # x-part first (independent of h), then h-part accumulates
                        nc.tensor.matmul(
                            ps[:],
                            wvar_sb[32:64, ts(j * 2 + p, 128)],
                            st[q][32:64, :],
                            start=True,
                            stop=False,
                        )
                        nc.tensor.matmul(
                            ps[:],
                            wvar_sb[0:32, ts(j * 2 + p, 128)],
                            st[q][0:32, :],
                            start=False,
                            stop=True,
                        )
                    else:
                        nc.tensor.matmul(
                            ps[:],
                            wvar_sb[:, ts(j * 2 + p, 128)],
                            st[q][:],
                            start=True,
                            stop=True,
                        )
                    s = work.tile([128, bc], f16, tag=f"s{q}", name=f"s{q}")
                    nc.scalar.activation(
                        s[:], ps[:], AF.Sigmoid, bias=bias_sb[:, p : p + 1]
                    )
                    sv[q] = s

                def ph_gc_t1(q):
                    p = PAT[q]
                    bd = BAND[p]
                    cb = 32 * p
                    bc = CSZ[q]
                    s = sv[q]
                    e_t1 = nc.gpsimd if ENG[q]["t1"] == "g" else nc.vector
                    e_gc = nc.gpsimd if ENG[q]["gc"] == "g" else nc.vector
                    gb = bd["i"]
                    gc = work.tile([96, bc], f16, tag=f"gc{q}", name=f"gc{q}")
                    e_gc.tensor_scalar(
                        gc[gb : gb + 32, :],
                        s[bd["g"] : bd["g"] + 32, :],
                        0.5,
                        None,
                        OP.subtract,
                    )
                    t1 = work.tile([64, bc], f16, tag=f"t1{q}", name=f"t1{q}")
                    e_t1.tensor_tensor(
                        t1[cb : cb + 32, :],
                        s[gb : gb + 32, :],
                        gc[gb : gb + 32, :],
                        OP.mult,
                    )
                    t1v[q] = t1

                def ph_c(q):
                    p = PAT[q]
                    bd = BAND[p]
                    cb = 32 * p
                    s = sv[q]
                    e_c = nc.gpsimd if ENG[q]["c"] == "g" else nc.vector
                    fb = bd["f"]
                    if PINGPONG:
                        cnew = work.tile(
                            [64, CSZ[q]], f16, tag=f"ct{q}", name=f"ct{q}"
                        )[cb : cb + 32, :]
                    else:
                        cnew = ctsl[q]
                    e_c.tensor_tensor(cnew[:], s[fb : fb + 32, :], ctsl[q][:], OP.mult)
                    e_c.tensor_tensor(cnew[:], cnew[:], t1v[q][cb : cb + 32, :], OP.add)
                    ctsl[q] = cnew

                def ph_tanh(q):
                    p = PAT[q]
                    bc = CSZ[q]
                    tb = 64 + 32 * p
                    th = work.tile([128, bc], f16, tag=f"th{q}", name=f"th{q}")
                    nc.scalar.activation(
                        th[tb : tb + 32, :], ctsl[q][:], AF.Tanh, scale=2.0
                    )
                    thv[q] = th

                def ph_h(q):
                    p = PAT[q]
                    bd = BAND[p]
                    bc = CSZ[q]
                    tb = 64 + 32 * p
                    s, th = sv[q], thv[q]
                    ob = bd["o"]
                    if t == t_steps - 1:
                        hf = work.tile([32, bc], f32, tag=f"hf{q}", name=f"hf{q}")
                        nc.vector.tensor_tensor(
                            hf[:], s[ob : ob + 32, :], th[tb : tb + 32, :], OP.mult
                        )
                        hfs[q] = hf
                    else:
                        nc.vector.tensor_tensor(
                            st[q][0:32, :], s[ob : ob + 32, :], th[tb : tb + 32, :],
                            OP.mult,
                        )

                qo = list(range(G))
                if EMIT == "rot":
                    r = t % G
                    qo = qo[r:] + qo[:r]
                if EMIT in ("chain", "rot"):
                    for q in qo:
                        ph_mm(q); ph_gc_t1(q); ph_c(q); ph_tanh(q); ph_h(q)
                else:
                    for q in qo: ph_mm(q)
                    for q in qo: ph_gc_t1(q)
                    for q in qo: ph_c(q)
                    for q in qo: ph_tanh(q)
                    for q in qo: ph_h(q)

            pso = psum.tile([1, B], f32, tag="ps0" if G >= 4 else "pso", bufs=None if G >= 4 else 1)
            for q in range(G):
                nc.tensor.matmul(
                    pso[0:1, COFF[q] : COFF[q] + CSZ[q]],
                    wout_sb[:, 0:1],
                    hfs[q][:],
                    start=True,
                    stop=True,
                )
            outs = work.tile([1, B], f32, tag="outs")
            nc.scalar.activation(outs[:], pso[:], AF.Identity, bias=bout_sb[:])
            nc.sync.dma_start(out_d[:], outs[:])
            if DUMMY_MM:
                scro = work.tile([1, 128], f32, tag="scro")
                nc.vector.tensor_copy(scro[:], dps[0:1, :])
                nc.sync.dma_start(scr_d[:], scro[:])

    nc.compile()
    return nc


def get_nc(t_steps=T):
    key = ("nc", t_steps)
    if key not in _cache:
        _cache[key] = _build(t_steps)
    return _cache[key]


def pack_inputs(input_seq, W_ih, W_hh, b_ih, b_hh, W_out, b_out):
    """Host-side layout prep: per-pattern gate reorder, g-row 2x prescale,
    lhsT window variants, per-core batch shards of x pre-transposed [T, B]."""
    W_hh = np.asarray(W_hh, np.float32)
    W_ih = np.asarray(W_ih, np.float32)
    bsum = np.asarray(b_ih, np.float32) + np.asarray(b_hh, np.float32)

    wvar = np.zeros((K, 2 * NW * 128), np.float32)
    bias = np.zeros((128, 2), np.float32)
    for p in range(2):
        perm = PERMS[p]
        gb = BAND[p]["g"]
        Wr = W_hh[perm].copy()           # [128, 32]
        wih = W_ih[perm, 0].copy()       # [128]
        br = bsum[perm].copy()
        Wr[gb : gb + 32] *= 2.0          # g rows: sigmoid(2a) trick
        wih[gb : gb + 32] *= 2.0
        br[gb : gb + 32] *= 2.0
        base = np.zeros((K, 128), np.float32)
        base[0:32] = Wr.T
        for jj in range(NW):
            blk = base.copy()
            blk[32 + jj, :] = wih
            wvar[:, (jj * 2 + p) * 128 : (jj * 2 + p + 1) * 128] = blk
        bias[:, p] = br

    wout = np.ascontiguousarray(
        np.asarray(W_out, np.float32).reshape(1, H).T
    )  # [32, 1]
    bout = np.asarray(b_out, np.float32).reshape(1, 1)

    x = np.asarray(input_seq, np.float32)[:, :, 0]  # [4096, 512]
    in_maps = []
    for c in range(NCORES):
        xT = np.ascontiguousarray(x[c * B : (c + 1) * B, :].T)  # [T, B]
        in_maps.append(
            {
                "xT": xT.astype(np.float16),
                "wvar": wvar.astype(np.float16),
                "bias": bias,
                "wout": wout,
                "bout": bout,
            }
        )
    return in_maps


def run(inputs: dict, trace: bool = False):
    """Run on all 8 NeuronCores. Returns (out [4096,1] f32, exec_time_ns|None)."""
    from concourse.bass_utils import run_bass_kernel_spmd

    nc = get_nc()
    in_maps = pack_inputs(**inputs)
    res = run_bass_kernel_spmd(nc, in_maps, core_ids=list(range(NCORES)), trace=trace)
    out = np.concatenate(
        [np.asarray(res.results[c]["out"], np.float32).reshape(-1) for c in range(NCORES)]
    ).reshape(BTOT, 1)
    return out, res.exec_time_ns


def kernel(**inputs) -> np.ndarray:
    out, _ = run(inputs)
    return out

